# revision 1
# baseline (speedup 1.0000x reference)
"""TRN2 Bass kernel for nn_CausalSelfAttention_63058709840004.

Sharding: tensor-parallel over heads (2 groups x 3 heads) x 4 causal query
shards = 8 cores. Each core computes K,V for its 3 heads over the full
sequence (replicated within the group), Q for its 1024 query rows (two
512-row chunks at s*512 and (s+4)*512), runs causal attention, and a partial
c_proj; the host sums the two head-group partials per row.

All matmuls run as float32r (TF32-like, 1 cyc/row at N>=256, ~1.6e-4 rel
err). rms-norm + rotary are done in natural [t, d] layout (free-axis
reductions + per-partition scales), then Q/K are PE-transposed to [d, t] for
the attention matmuls. Softmax skips max-subtraction (|scores| <= 15.4
bounded by Cauchy-Schwarz after rms-norm, safe in fp32).
"""
import numpy as np

import concourse.bass as bass
import concourse.bacc as bacc
import concourse.mybir as mybir
import concourse.tile as tile
from concourse.bass_utils import run_bass_kernel_spmd

T, DIM, H, D = 4096, 768, 6, 128
HPG = 3  # heads per group
GDIM = HPG * D  # 384
ATTN_SCALE = 0.12
EPS = 1.1920929e-07
NT = T // 128  # 32 t-tiles
NQ = 1024 // 128  # 8 q-tiles per core
F32 = mybir.dt.float32
F32R = mybir.dt.float32r
U32 = mybir.dt.uint32
EXP = mybir.ActivationFunctionType.Exp
SQUARE = mybir.ActivationFunctionType.Square
SQRT = mybir.ActivationFunctionType.Sqrt
MASK_NEG = -1.0e5

_CACHE = {}


def _rotary(nc, pool, nat, cos_b, sin_p, sin_n, nh):
    """In-place rotary on nat [128, nh, 128] (scaled). Rotates dim pairs
    (i, 64+i) for i in 0..31 (freqs 32..63 are zero -> identity)."""
    x1 = nat[:, :, 0:32]
    x2 = nat[:, :, 64:96]
    ta = pool.tile([128, nh, 32], F32R, tag="rot_ta")
    tb = pool.tile([128, nh, 32], F32R, tag="rot_tb")
    ua = pool.tile([128, nh, 32], F32R, tag="rot_ua")
    ub = pool.tile([128, nh, 32], F32R, tag="rot_ub")
    nc.vector.tensor_mul(out=ta[:], in0=x2, in1=sin_p)  # x2*sin
    nc.vector.tensor_mul(out=tb[:], in0=x1, in1=sin_n)  # -x1*sin
    nc.vector.tensor_mul(out=ua[:], in0=x1, in1=cos_b)  # x1*cos
    nc.vector.tensor_mul(out=ub[:], in0=x2, in1=cos_b)  # x2*cos
    nc.vector.tensor_add(out=x1, in0=ua[:], in1=ta[:])  # y1 = x1*c + x2*s
    nc.vector.tensor_add(out=x2, in0=ub[:], in1=tb[:])  # y2 = x2*c - x1*s


def build_nc(variant=None):
    nc = bacc.Bacc(None, target_bir_lowering=False)

    # ---- DRAM tensors (per-core inputs prepared by the host) ----
    xTt = nc.dram_tensor("xTt", [NT // 2, 128, 6, 2, 128], F32R, kind="ExternalInput")
    xqTt = nc.dram_tensor("xqTt", [NQ // 2, 128, 6, 2, 128], F32R, kind="ExternalInput")
    wkv = nc.dram_tensor("wkv", [128, 6, 2 * GDIM], F32R, kind="ExternalInput")
    wq = nc.dram_tensor("wq", [128, 6, GDIM], F32R, kind="ExternalInput")
    vek = nc.dram_tensor("vek", [NT // 2, 128, 2, GDIM], F32, kind="ExternalInput")
    cosk = nc.dram_tensor("cosk", [128, NT, 32], F32, kind="ExternalInput")
    sinkpm = nc.dram_tensor("sinkpm", [128, NT, 64], F32, kind="ExternalInput")
    cosq = nc.dram_tensor("cosq", [128, NQ, 32], F32, kind="ExternalInput")
    sinqpm = nc.dram_tensor("sinqpm", [128, NQ, 64], F32, kind="ExternalInput")
    cprojT = nc.dram_tensor("cprojT", [128, HPG, DIM], F32R, kind="ExternalInput")
    ident_in = nc.dram_tensor("ident", [128, 128], F32R, kind="ExternalInput")
    ones_col_in = nc.dram_tensor("ones_col", [128, 1], F32R, kind="ExternalInput")
    ones_row_in = nc.dram_tensor("ones_row", [1, 128], F32R, kind="ExternalInput")
    svar_t = nc.dram_tensor("svar", [1, 1], U32, kind="ExternalInput")
    y_out = nc.dram_tensor("y", [1024, DIM], F32, kind="ExternalOutput")


    with tile.TileContext(nc) as tc:
        # core-variant register (s = core % 4)
        tmp = nc.alloc_registers("tmp_svar", mybir.ALL_ENGINES)
        nc.regs_load(tmp, svar_t[0:1, 0:1])
        sv = nc.snap(tmp, donate=True, min_val=0, max_val=3)

        with tc.tile_pool(name="res", bufs=1) as res:
            KT = res.tile([128, HPG, T], F32R, tag="KT")
            Vn = res.tile([128, NT, GDIM], F32R, tag="Vn")
            QT = res.tile([128, HPG, 1024], F32R, tag="QT")
            Ysb = res.tile([128, HPG, 1024], F32R, tag="Ysb")
            cproj_sb = res.tile([128, HPG, DIM], F32R, tag="cproj")
            ident = res.tile([128, 128], F32R, tag="ident")
            ones_col = res.tile([128, 1], F32R, tag="ones_col")
            ones_row = res.tile([1, 128], F32R, tag="ones_row")
            nc.gpsimd.dma_start(ident[:], ident_in[:])
            nc.gpsimd.dma_start(ones_col[:], ones_col_in[:])
            nc.gpsimd.dma_start(ones_row[:], ones_row_in[:])
            eps_k = res.tile([128, 1], F32, tag="eps_k")
            eps_q = res.tile([128, 1], F32, tag="eps_q")
            nc.gpsimd.memset(eps_k[:], EPS)
            nc.gpsimd.memset(eps_q[:], EPS / (ATTN_SCALE * ATTN_SCALE))

            # ================= Phase A/B: projections =================
            with (
                tc.tile_pool(name="wp", bufs=1) as wp,
                tc.tile_pool(name="ap", bufs=3) as ap,
                tc.tile_pool(name="st", bufs=3) as st,
                tc.tile_pool(name="rot", bufs=1) as rot,
                tc.tile_pool(name="pp", bufs=3, space="PSUM") as pp,
                tc.tile_pool(name="pt", bufs=2, space="PSUM") as pt,
            ):
                wkv_sb = wp.tile([128, 6, 2 * GDIM], F32R, tag="wkv")
                wq_sb = wp.tile([128, 6, GDIM], F32R, tag="wq")
                cosk_sb = wp.tile([128, NT, 32], F32, tag="cosk")
                sinkpm_sb = wp.tile([128, NT, 64], F32, tag="sinkpm")
                cosq_sb = wp.tile([128, NQ, 32], F32, tag="cosq")
                sinqpm_sb = wp.tile([128, NQ, 64], F32, tag="sinqpm")
                for md in range(6):
                    nc.scalar.dma_start(wkv_sb[:, md], wkv[:, md])
                    nc.scalar.dma_start(wq_sb[:, md], wq[:, md])
                nc.gpsimd.dma_start(cosk_sb[:], cosk[:])
                nc.gpsimd.dma_start(sinkpm_sb[:], sinkpm[:])
                nc.gpsimd.dma_start(cosq_sb[:], cosq[:])
                nc.gpsimd.dma_start(sinqpm_sb[:], sinqpm[:])

                # ---- Phase A: K,V over full sequence (two t-tiles per iter;
                # xt via SP HWDGE, ve via gpsimd SWDGE to spread dispatch) ----
                for tp in range(NT // 2):
                    xt2 = st.tile([128, 6, 2, 128], F32R, tag="xt")
                    nc.sync.dma_start(xt2[:], xTt[tp])
                    vet2 = st.tile([128, 2, GDIM], F32, tag="vet")
                    nc.gpsimd.dma_start(vet2[:], vek[tp])
                    for u in range(2):
                        ti = 2 * tp + u
                        k_ps = pp.tile([128, GDIM], F32, tag="k_ps")
                        v_ps = pp.tile([128, GDIM], F32, tag="v_ps")
                        for md in range(6):
                            nc.tensor.matmul(
                                k_ps[:], xt2[:, md, u], wkv_sb[:, md, 0:GDIM],
                                start=(md == 0), stop=(md == 5), skip_group_check=True,
                            )
                            nc.tensor.matmul(
                                v_ps[:], xt2[:, md, u], wkv_sb[:, md, GDIM : 2 * GDIM],
                                start=(md == 0), stop=(md == 5), skip_group_check=True,
                            )
                        # V: add pre-scaled ve, store natural
                        nc.vector.tensor_add(out=Vn[:, ti, :], in0=v_ps[:], in1=vet2[:, u])
                        # K: rms-norm scale b = 1/sqrt(mean(k^2)+eps) per row/head
                        ssq = ap.tile([128, HPG], F32, tag="ssq")
                        scratch = ap.tile([128, D], F32, tag="scratch")
                        for h in range(HPG):
                            nc.scalar.activation(
                                scratch[:], k_ps[:, h * D : (h + 1) * D],
                                SQUARE, accum_out=ssq[:, h : h + 1],
                            )
                        bsc = ap.tile([128, HPG], F32, tag="bsc")
                        nc.scalar.activation(bsc[:], ssq[:], SQRT, bias=eps_k[:], scale=1.0 / D)
                        nc.vector.reciprocal(bsc[:], bsc[:])
                        knat = ap.tile([128, HPG, D], F32R, tag="knat")
                        nc.vector.tensor_mul(
                            out=knat[:],
                            in0=k_ps[:].rearrange("p (h d) -> p h d", d=D),
                            in1=bsc[:, :, None].to_broadcast((128, HPG, D)),
                        )
                        _rotary(
                            nc, rot, knat,
                            cosk_sb[:, ti, None, :].to_broadcast((128, HPG, 32)),
                            sinkpm_sb[:, ti, None, 0:32].to_broadcast((128, HPG, 32)),
                            sinkpm_sb[:, ti, None, 32:64].to_broadcast((128, HPG, 32)),
                            HPG,
                        )
                        tr = pt.tile([128, GDIM], F32R, tag="tr")
                        for h in range(HPG):
                            nc.tensor.transpose(tr[:, h * D : (h + 1) * D], knat[:, h], ident[:])
                        nc.vector.tensor_copy(
                            KT[:, :, ti * 128 : (ti + 1) * 128],
                            tr[:].rearrange("p (h d) -> p h d", d=D),
                        )

                # ---- Phase B: Q over this core's 1024 rows ----
                for tp in range(NQ // 2):
                  xt2 = st.tile([128, 6, 2, 128], F32R, tag="xt")
                  nc.sync.dma_start(xt2[:], xqTt[tp])
                  for u in range(2):
                    ti = 2 * tp + u
                    q_ps = pp.tile([128, GDIM], F32, tag="k_ps")
                    for md in range(6):
                        nc.tensor.matmul(
                            q_ps[:], xt2[:, md, u], wq_sb[:, md],
                            start=(md == 0), stop=(md == 5), skip_group_check=True,
                        )
                    ssq = ap.tile([128, HPG], F32, tag="ssq")
                    scratch = ap.tile([128, D], F32, tag="scratch")
                    for h in range(HPG):
                        nc.scalar.activation(
                            scratch[:], q_ps[:, h * D : (h + 1) * D],
                            SQUARE, accum_out=ssq[:, h : h + 1],
                        )
                    # a = ATTN_SCALE / sqrt(mean+eps) = 1/sqrt((m/D+eps)/s^2)
                    asc = ap.tile([128, HPG], F32, tag="bsc")
                    s2 = ATTN_SCALE * ATTN_SCALE
                    nc.scalar.activation(asc[:], ssq[:], SQRT, bias=eps_q[:], scale=1.0 / (D * s2))
                    nc.vector.reciprocal(asc[:], asc[:])
                    qnat = ap.tile([128, HPG, D], F32R, tag="knat")
                    nc.vector.tensor_mul(
                        out=qnat[:],
                        in0=q_ps[:].rearrange("p (h d) -> p h d", d=D),
                        in1=asc[:, :, None].to_broadcast((128, HPG, D)),
                    )
                    _rotary(
                        nc, rot, qnat,
                        cosq_sb[:, ti, None, :].to_broadcast((128, HPG, 32)),
                        sinqpm_sb[:, ti, None, 0:32].to_broadcast((128, HPG, 32)),
                        sinqpm_sb[:, ti, None, 32:64].to_broadcast((128, HPG, 32)),
                        HPG,
                    )
                    tr = pt.tile([128, GDIM], F32R, tag="tr")
                    for h in range(HPG):
                        nc.tensor.transpose(tr[:, h * D : (h + 1) * D], qnat[:, h], ident[:])
                    nc.vector.tensor_copy(
                        QT[:, :, ti * 128 : (ti + 1) * 128],
                        tr[:].rearrange("p (h d) -> p h d", d=D),
                    )

            nc.gpsimd.dma_start(cproj_sb[:], cprojT[:])

            # ================= Phase D (emitted inside each variant) ======
            def cproj(s):
                with (
                    tc.tile_pool(name=f"op{s}", bufs=3) as op,
                    tc.tile_pool(name=f"psO{s}", bufs=3, space="PSUM") as psO,
                ):
                    for m in range(NQ):
                        o_sb = op.tile([128, DIM], F32, tag="o_sb")
                        for oc in range(2):
                            o_ps = psO.tile([128, GDIM], F32, tag="o_ps")
                            for h in range(HPG):
                                nc.tensor.matmul(
                                    o_ps[:],
                                    Ysb[:, h, m * 128 : (m + 1) * 128],
                                    cproj_sb[:, h, oc * GDIM : (oc + 1) * GDIM],
                                    start=(h == 0), stop=(h == 2), skip_group_check=True,
                                )
                            nc.vector.tensor_copy(o_sb[:, oc * GDIM : (oc + 1) * GDIM], o_ps[:])
                        nc.sync.dma_start(y_out[m * 128 : (m + 1) * 128, :], o_sb[:])

            # ================= Phase C: attention (per-core variant) ======
            def attention(s):
                with (
                    tc.tile_pool(name=f"ep{s}", bufs=6) as ep,
                    tc.tile_pool(name=f"rp{s}", bufs=2) as rp,
                    tc.tile_pool(name=f"psS{s}", bufs=4, space="PSUM") as psS,
                    tc.tile_pool(name=f"psY{s}", bufs=2, space="PSUM") as psY,
                    tc.tile_pool(name=f"psD{s}", bufs=2, space="PSUM") as psD,
                ):
                    # paired causal chunks (s, 7-s): 36 key-tiles total on every core
                    chunks = [(0, 4 * s + 4), (512, 32 - 4 * s)]
                    for h in range(HPG):
                        for qoff, nk in chunks:
                            y_ps = psY.tile([128, 512], F32, tag="y")
                            d_ps = psD.tile([1, 512], F32, tag="d")
                            for k in range(nk):
                                s_ps = psS.tile([128, 512], F32, tag="s")
                                nc.tensor.matmul(
                                    s_ps[:],
                                    KT[:, h, k * 128 : (k + 1) * 128],
                                    QT[:, h, qoff : qoff + 512],
                                    start=True, stop=True, skip_group_check=True,
                                )
                                E = ep.tile([128, 512], F32R, tag="E")
                                nc.scalar.activation(E[:], s_ps[:], EXP)
                                i = k - (nk - 4)
                                if i >= 0:
                                    # zero E where key row 128*i+r > query col
                                    nc.gpsimd.affine_select(
                                        out=E[:], in_=E[:],
                                        compare_op=mybir.AluOpType.is_ge,
                                        fill=0.0, base=-128 * i,
                                        pattern=[[1, 512]], channel_multiplier=-1,
                                    )
                                nc.tensor.matmul(
                                    d_ps[:], ones_col[:], E[:],
                                    start=(k == 0), stop=(k == nk - 1),
                                    skip_group_check=True,
                                )
                                nc.tensor.matmul(
                                    y_ps[:], Vn[:, k, h * D : (h + 1) * D], E[:],
                                    start=(k == 0), stop=(k == nk - 1),
                                    skip_group_check=True,
                                )
                            recip = rp.tile([1, 512], F32R, tag="recip")
                            with nc.allow_low_precision(
                                reason="1/denom as f32r; ~1e-4 uniform scale wobble"
                            ):
                                nc.vector.reciprocal(recip[:], d_ps[:])
                            bc = rp.tile([128, 512], F32R, tag="bc")
                            nc.gpsimd.partition_broadcast(bc[:], recip[0:1, :])
                            ysl = Ysb[:, h, qoff : qoff + 512]
                            nc.vector.tensor_copy(ysl, y_ps[:])
                            nc.vector.tensor_mul(out=ysl, in0=ysl, in1=bc[:])
                cproj(s)

            if variant is not None:
                attention(variant)
            else:
                with tc.If(sv == 0) as c0:
                    attention(0)
                with c0.Else():
                    with tc.If(sv == 1) as c1:
                        attention(1)
                    with c1.Else():
                        with tc.If(sv == 2) as c2:
                            attention(2)
                        with c2.Else():
                            attention(3)


    nc.finalize()
    return nc


def _host_prep(x, ve, qkv_w, lambdas, c_proj_w):
    """Build the 8 per-core input maps."""
    x2d = np.ascontiguousarray(x.reshape(T, DIM), dtype=np.float32)
    xT = np.ascontiguousarray(x2d.T)
    ve2 = ve.reshape(T, H, D).astype(np.float32)
    lam0, lam1 = float(lambdas[0]), float(lambdas[1])
    wq_all, wk_all, wv_all = qkv_w[0], qkv_w[1], qkv_w[2]  # [768, 768] each

    t = np.arange(T, dtype=np.float32)
    af = (1.0 / 1024.0) ** np.linspace(0.0, 1.0, 32, dtype=np.float32)
    theta = t[:, None] * af[None, :]
    cos_t = np.cos(theta).astype(np.float32)  # [T, 32]
    sin_t = np.sin(theta).astype(np.float32)
    sin_pm = np.concatenate([sin_t, -sin_t], axis=1)  # [T, 64]

    ident = np.eye(128, dtype=np.float32)
    ones_col = np.ones((128, 1), dtype=np.float32)
    ones_row = np.ones((1, 128), dtype=np.float32)

    def pack_xT(m):  # [768, t] -> [t/256, 128, 6, 2, 128]
        t = m.shape[1]
        return np.ascontiguousarray(
            m.reshape(6, 128, t // 256, 2, 128).transpose(2, 1, 0, 3, 4)
        )

    def pack_rows(m):  # [t, d] -> [t/256, 128, 2, d]
        t, d = m.shape
        return np.ascontiguousarray(m.reshape(t // 256, 2, 128, d).transpose(0, 2, 1, 3))

    def pack_tab(m):  # [t, c] -> [128, t/128, c]
        t, c = m.shape
        return np.ascontiguousarray(m.reshape(t // 128, 128, c).transpose(1, 0, 2))

    xT_packed = pack_xT(xT)
    cosk_p = pack_tab(cos_t)
    sinkpm_p = pack_tab(sin_pm)
    in_maps = []
    for c in range(8):
        g, s = divmod(c, 4)
        hsl = slice(g * GDIM, (g + 1) * GDIM)
        qrows = np.r_[512 * s : 512 * (s + 1), 512 * (7 - s) : 512 * (8 - s)]
        wkv = np.concatenate([wk_all[hsl], lam0 * wv_all[hsl]], axis=0)  # [768, 768]
        in_maps.append(
            {
                "xTt": xT_packed,
                "xqTt": pack_xT(np.ascontiguousarray(xT[:, qrows])),
                "wkv": np.ascontiguousarray(
                    wkv.T.astype(np.float32).reshape(6, 128, 768).transpose(1, 0, 2)
                ),
                "wq": np.ascontiguousarray(
                    wq_all[hsl].T.astype(np.float32).reshape(6, 128, GDIM).transpose(1, 0, 2)
                ),
                "vek": pack_rows(
                    (lam1 * ve2[:, g * HPG : (g + 1) * HPG, :]).reshape(T, GDIM)
                ),
                "cosk": cosk_p,
                "sinkpm": sinkpm_p,
                "cosq": pack_tab(np.ascontiguousarray(cos_t[qrows])),
                "sinqpm": pack_tab(np.ascontiguousarray(sin_pm[qrows])),
                "cprojT": np.ascontiguousarray(
                    c_proj_w[:, hsl].T.astype(np.float32).reshape(HPG, 128, DIM).transpose(1, 0, 2)
                ),
                "ident": ident,
                "ones_col": ones_col,
                "ones_row": ones_row,
                "svar": np.array([[s]], dtype=np.uint32),
            }
        )
    return in_maps


def run(inputs, **run_kwargs):
    if "nc" not in _CACHE:
        _CACHE["nc"] = build_nc()
    nc = _CACHE["nc"]
    in_maps = _host_prep(
        inputs["x"], inputs["ve"], inputs["qkv_w"], inputs["lambdas"], inputs["c_proj_w"]
    )
    res = run_bass_kernel_spmd(nc, in_maps, core_ids=list(range(8)), **run_kwargs)
    out = np.zeros((T, DIM), dtype=np.float32)
    for c, r in enumerate(res.results):
        s = c % 4
        y = r["y"]
        out[512 * s : 512 * (s + 1)] += y[:512]
        out[512 * (7 - s) : 512 * (8 - s)] += y[512:]
    return out.reshape(1, T, DIM), res


def kernel(**inputs):
    out, _ = run(inputs)
    return out



# revision 2
# speedup vs baseline: 11.4947x; 11.4947x over previous
"""TRN2 Bass kernel v2 for nn_CausalSelfAttention_63058709840004.

Sharding: tensor-parallel over heads (2 groups x 3 heads) x 4 causal query
shards = 8 cores. Each core computes K,V for its 3 heads over the full
sequence (replicated within the group), Q for its 1024 query rows (two
512-row chunks at s*512 and (s+4)*512), runs causal attention, and a partial
c_proj; the host sums the two head-group partials per row.

v2 vs v1:
- bf16 data path (x, ve, weights, KT/Vn/QT/E/Ysb, rotary tables): halves HBM
  traffic, 2x DVE on all-SBUF bf16 ops, 1.0 cyc/row PE transposes.
- rms-norm sum-of-squares on DVE (square + tensor_reduce) instead of 3
  ACT SQUARE+accum calls; ACT does the PSUM->SBUF copy instead.
- single interleaved schedule: Q tiles first (split QT per q-chunk), then
  K/V pairs with attention iterations pumped in as soon as their key tiles
  are emitted. Attention units (chunk, head) run small-chunk-first; scores
  run one key-tile ahead of the y/d accumulation (software pipeline) so the
  in-order PE never waits on the exp.
- PSUM fits 8 banks concurrently: k/v/tr bufs=1/1/1 (3) + scores 2 +
  y 2 + d 1 = 8; cproj's o_ps pool opens after the projection pools close.
- cproj(chunk0) emitted shortly after the last K/V pair; cproj(chunk1) tail.
"""
from contextlib import ExitStack

import numpy as np

import concourse.bass as bass
import concourse.bacc as bacc
import concourse.mybir as mybir
import concourse.tile as tile
from concourse.bass_utils import run_bass_kernel_spmd

# The activation-table pass maps each function to the FIRST table set that
# contains it, so Ln (set "natural_log") and Exp (set "exp_and_others")
# alternating in one stream reload the table every call (~1.3us each).
# The set index is the runtime act_func_set_id (must stay canonical), so
# instead of reordering, strip Exp/Ln/Copy/Square from every set BEFORE the
# combined "natural_log_exp_and_others" set: the first (and only) match for
# all functions this kernel uses is then that one set -> one table load.
_orig_get_activation_tables = bacc.get_activation_tables


def _get_activation_tables_one_set(arch):
    tabs = _orig_get_activation_tables(arch)
    key = "natural_log_exp_and_others"
    if key not in tabs:
        return tabs
    mine = {
        mybir.ActivationFunctionType.Exp,
        mybir.ActivationFunctionType.Ln,
        mybir.ActivationFunctionType.Copy,
        mybir.ActivationFunctionType.Square,
    }
    assert mine <= set(tabs[key])
    out = {}
    for name, funcs in tabs.items():
        out[name] = set(funcs) if name == key else set(funcs) - mine
    return out


bacc.get_activation_tables = _get_activation_tables_one_set

T, DIM, H, D = 4096, 768, 6, 128
HPG = 3  # heads per group
GDIM = HPG * D  # 384
ATTN_SCALE = 0.12
EPS = 1.1920929e-07
NT = T // 128  # 32 t-tiles
NQ = 1024 // 128  # 8 q-tiles per core
F32 = mybir.dt.float32
F32R = mybir.dt.float32r
BF16 = mybir.dt.bfloat16
U32 = mybir.dt.uint32
EXP = mybir.ActivationFunctionType.Exp
LN = mybir.ActivationFunctionType.Ln
COPY = mybir.ActivationFunctionType.Copy
AX_X = mybir.AxisListType.X
ADD = mybir.AluOpType.add

_CACHE = {}


def _rotary(nc, pool, nat, cos_b, sin_p, sin_n, nh):
    """In-place rotary on nat [128, nh, 128] bf16. Rotates dim pairs
    (i, 64+i) for i in 0..31 (freqs 32..63 are zero -> identity)."""
    x1 = nat[:, :, 0:32]
    x2 = nat[:, :, 64:96]
    ta = pool.tile([128, nh, 32], BF16, tag="rot_ta")
    tb = pool.tile([128, nh, 32], BF16, tag="rot_tb")
    ua = pool.tile([128, nh, 32], BF16, tag="rot_ua")
    ub = pool.tile([128, nh, 32], BF16, tag="rot_ub")
    with nc.allow_low_precision(reason="bf16 rotary; ~4e-3 rel on rotation"):
        nc.vector.tensor_mul(out=ta[:], in0=x2, in1=sin_p)  # x2*sin
        nc.vector.tensor_mul(out=tb[:], in0=x1, in1=sin_n)  # -x1*sin
        nc.vector.tensor_mul(out=ua[:], in0=x1, in1=cos_b)  # x1*cos
        nc.vector.tensor_mul(out=ub[:], in0=x2, in1=cos_b)  # x2*cos
        nc.vector.tensor_add(out=x1, in0=ua[:], in1=ta[:])  # y1 = x1*c + x2*s
        nc.vector.tensor_add(out=x2, in0=ub[:], in1=tb[:])  # y2 = x2*c - x1*s


def build_nc(variant=None):
    nc = bacc.Bacc(None, target_bir_lowering=False)

    # ---- DRAM tensors (per-core inputs prepared by the host, bf16) ----
    xTt = nc.dram_tensor("xTt", [NT // 2, 128, 6, 2, 128], BF16, kind="ExternalInput")
    xqTt = nc.dram_tensor("xqTt", [NQ // 2, 128, 6, 2, 128], BF16, kind="ExternalInput")
    wkv = nc.dram_tensor("wkv", [128, 6, 2 * GDIM], BF16, kind="ExternalInput")
    wq = nc.dram_tensor("wq", [128, 6, GDIM], BF16, kind="ExternalInput")
    vek = nc.dram_tensor("vek", [NT // 2, 128, 2, GDIM], BF16, kind="ExternalInput")
    cosk = nc.dram_tensor("cosk", [128, NT, 32], BF16, kind="ExternalInput")
    sinkpm = nc.dram_tensor("sinkpm", [128, NT, 64], BF16, kind="ExternalInput")
    cosq = nc.dram_tensor("cosq", [128, NQ, 32], BF16, kind="ExternalInput")
    sinqpm = nc.dram_tensor("sinqpm", [128, NQ, 64], BF16, kind="ExternalInput")
    cprojT = nc.dram_tensor("cprojT", [128, HPG, DIM], BF16, kind="ExternalInput")
    ident_in = nc.dram_tensor("ident", [128, 128], BF16, kind="ExternalInput")
    ones_mat_in = nc.dram_tensor("ones_mat", [128, 128], BF16, kind="ExternalInput")
    svar_t = nc.dram_tensor("svar", [1, 1], U32, kind="ExternalInput")
    y_out = nc.dram_tensor("y", [1024, DIM], F32, kind="ExternalOutput")

    with tile.TileContext(nc) as tc:
        # core-variant register (s = core % 4)
        tmp = nc.alloc_registers("tmp_svar", mybir.ALL_ENGINES)
        nc.regs_load(tmp, svar_t[0:1, 0:1])
        sv = nc.snap(tmp, donate=True, min_val=0, max_val=3)

        with tc.tile_pool(name="res", bufs=1) as res:
            KT = [
                res.tile([128, HPG, 128], BF16, tag=f"KT{k}", name=f"KT{k}")
                for k in range(NT)
            ]
            Vn = [
                res.tile([128, GDIM], BF16, tag=f"Vn{k}", name=f"Vn{k}")
                for k in range(NT)
            ]
            QTc = [
                res.tile([128, HPG, 512], BF16, tag=f"QTc{c}", name=f"QTc{c}")
                for c in range(2)
            ]
            Ysb = [
                res.tile([128, HPG, 512], BF16, tag=f"Ysb{c}", name=f"Ysb{c}")
                for c in range(2)
            ]
            cproj_sb = res.tile([128, HPG, DIM], BF16, tag="cproj")
            ident = res.tile([128, 128], BF16, tag="ident")
            ones_mat = res.tile([128, 128], BF16, tag="ones_mat")
            nc.gpsimd.dma_start(ident[:], ident_in[:])
            nc.gpsimd.dma_start(ones_mat[:], ones_mat_in[:])
            nc.gpsimd.dma_start(cproj_sb[:], cprojT[:])
            eps_k = res.tile([128, 1], F32, tag="eps_k")
            eps_q = res.tile([128, 1], F32, tag="eps_q")
            nc.gpsimd.memset(eps_k[:], EPS)
            nc.gpsimd.memset(eps_q[:], EPS / (ATTN_SCALE * ATTN_SCALE))

            def emit_all(s):
                """Whole per-core schedule for variant s (s = core%4)."""
                # attention pools open first so the projection pools (closed
                # at the cproj boundary) pop in proper LIFO order
                att_stack = ExitStack()
                ep = att_stack.enter_context(tc.tile_pool(name=f"ep{s}", bufs=6))
                rp = att_stack.enter_context(tc.tile_pool(name=f"rp{s}", bufs=2))
                psS = att_stack.enter_context(
                    tc.tile_pool(name=f"psS{s}", bufs=3, space="PSUM")
                )
                psY = att_stack.enter_context(
                    tc.tile_pool(name=f"psY{s}", bufs=2, space="PSUM")
                )
                psD = att_stack.enter_context(
                    tc.tile_pool(name=f"psD{s}", bufs=1, space="PSUM")
                )

                proj_stack = ExitStack()
                wp = proj_stack.enter_context(tc.tile_pool(name=f"wp{s}", bufs=1))
                ap = proj_stack.enter_context(tc.tile_pool(name=f"ap{s}", bufs=4))
                st = proj_stack.enter_context(tc.tile_pool(name=f"st{s}", bufs=3))
                rot = proj_stack.enter_context(tc.tile_pool(name=f"rot{s}", bufs=2))
                pk = proj_stack.enter_context(
                    tc.tile_pool(name=f"pk{s}", bufs=1, space="PSUM")
                )
                pv = proj_stack.enter_context(
                    tc.tile_pool(name=f"pv{s}", bufs=1, space="PSUM")
                )

                wkv_md = [
                    wp.tile([128, 2 * GDIM], BF16, tag=f"wkv{md}", name=f"wkv{md}")
                    for md in range(6)
                ]
                wq_md = [
                    wp.tile([128, GDIM], BF16, tag=f"wq{md}", name=f"wq{md}")
                    for md in range(6)
                ]
                cosk_sb = wp.tile([128, NT, 32], BF16, tag="cosk", name="cosk_sb")
                sinkpm_sb = wp.tile([128, NT, 64], BF16, tag="sinkpm", name="sinkpm_sb")
                cosq_sb = wp.tile([128, NQ, 32], BF16, tag="cosq", name="cosq_sb")
                sinqpm_sb = wp.tile([128, NQ, 64], BF16, tag="sinqpm", name="sinqpm_sb")
                for md in range(6):
                    nc.scalar.dma_start(wq_md[md][:], wq[:, md])
                    nc.scalar.dma_start(wkv_md[md][:], wkv[:, md])
                nc.gpsimd.dma_start(cosq_sb[:], cosq[:])
                nc.gpsimd.dma_start(sinqpm_sb[:], sinqpm[:])
                nc.gpsimd.dma_start(cosk_sb[:], cosk[:])
                nc.gpsimd.dma_start(sinkpm_sb[:], sinkpm[:])

                # ---------- projection helpers ----------
                pending_tr = []  # lag transposes one tile behind the matmuls

                def flush_tr(n=None):
                    while pending_tr and (n is None or len(pending_tr) > n):
                        pending_tr.pop(0)()

                def proj_tail(ps, cos_sb, sin_sb, ti, eps_t, scale, out_cb):
                    """Rms-norm + rotary + 3 PE transposes for one 128-row
                    projection tile sitting in PSUM (K or Q)."""
                    nat = ap.tile([128, HPG, D], BF16, tag="nat", name="nat")
                    nc.scalar.activation(
                        nat[:].rearrange("p h d -> p (h d)"), ps[:], COPY
                    )
                    _rotary(
                        nc, rot, nat,
                        cos_sb[:, ti, None, :].to_broadcast((128, HPG, 32)),
                        sin_sb[:, ti, None, 0:32].to_broadcast((128, HPG, 32)),
                        sin_sb[:, ti, None, 32:64].to_broadcast((128, HPG, 32)),
                        HPG,
                    )
                    sq = ap.tile([128, HPG, D], BF16, tag="sq", name="sq")
                    ssq = ap.tile([128, HPG], BF16, tag="ssq", name="ssq")
                    lnv = ap.tile([128, HPG], F32, tag="lnv", name="lnv")
                    bscb = ap.tile([128, HPG], BF16, tag="bsc", name="bscb")
                    with nc.allow_low_precision(reason="bf16 rms stats; ~4e-3"):
                        nc.vector.tensor_mul(out=sq[:], in0=nat[:], in1=nat[:])
                        nc.vector.tensor_reduce(ssq[:], sq[:], AX_X, ADD)
                    # rsqrt as exp(-0.5 * ln(x)): Ln/Exp/Copy share one ACT
                    # table, so the kernel never reloads the table mid-stream
                    nc.scalar.activation(lnv[:], ssq[:], LN, bias=eps_t[:],
                                         scale=scale)
                    nc.scalar.activation(bscb[:], lnv[:], EXP, scale=-0.5)
                    with nc.allow_low_precision(reason="bf16 rms scale; ~4e-3"):
                        nc.vector.tensor_mul(
                            out=nat[:], in0=nat[:],
                            in1=bscb[:, :, None].to_broadcast((128, HPG, D)),
                        )

                    def do_tr():
                        # shares the scores pool: during projections the pump
                        # runs skew-1, so at most 2 of the 3 "s" slots hold
                        # live score tiles and the third takes the transpose
                        tr = psS.tile([128, GDIM], BF16, tag="s", name="tr")
                        for h in range(HPG):
                            nc.tensor.transpose(
                                tr[:, h * D : (h + 1) * D], nat[:, h], ident[:]
                            )
                        out_cb(tr)

                    pending_tr.append(do_tr)

                def emit_q_pair(tp):
                    xt2 = st.tile([128, 6, 2, 128], BF16, tag="xt", name="xtq")
                    nc.sync.dma_start(xt2[:], xqTt[tp])
                    for u in range(2):
                        ti = 2 * tp + u
                        ci, tq = divmod(ti, 4)
                        q_ps = pk.tile([128, GDIM], F32, tag="k_ps", name="q_ps")
                        for md in range(6):
                            nc.tensor.matmul(
                                q_ps[:], xt2[:, md, u], wq_md[md][:],
                                start=(md == 0), stop=(md == 5), skip_group_check=True,
                            )

                        def store_q(tr, ci=ci, tq=tq):
                            with nc.allow_low_precision(reason="bf16 QT"):
                                nc.vector.tensor_copy(
                                    QTc[ci][:, :, tq * 128 : (tq + 1) * 128],
                                    tr[:].rearrange("p (h d) -> p h d", d=D),
                                )

                        proj_tail(q_ps, cosq_sb, sinqpm_sb, ti, eps_q,
                                  1.0 / (D * ATTN_SCALE * ATTN_SCALE), store_q)
                        flush_tr(2)

                def emit_kv_pair(tp):
                    xt2 = st.tile([128, 6, 2, 128], BF16, tag="xt", name="xtk")
                    nc.sync.dma_start(xt2[:], xTt[tp])
                    vet2 = st.tile([128, 2, GDIM], BF16, tag="vet", name="vet")
                    nc.gpsimd.dma_start(vet2[:], vek[tp])
                    for u in range(2):
                        ti = 2 * tp + u
                        k_ps = pk.tile([128, GDIM], F32, tag="k_ps", name="k_ps")
                        v_ps = pv.tile([128, GDIM], F32, tag="v_ps", name="v_ps")
                        for md in range(6):
                            nc.tensor.matmul(
                                k_ps[:], xt2[:, md, u], wkv_md[md][:, 0:GDIM],
                                start=(md == 0), stop=(md == 5), skip_group_check=True,
                            )
                            nc.tensor.matmul(
                                v_ps[:], xt2[:, md, u], wkv_md[md][:, GDIM : 2 * GDIM],
                                start=(md == 0), stop=(md == 5), skip_group_check=True,
                            )
                        # V first on DVE so v_ps releases quickly (pv bufs=1)
                        with nc.allow_low_precision(reason="bf16 V; ~4e-3"):
                            nc.vector.tensor_add(
                                out=Vn[ti][:], in0=v_ps[:], in1=vet2[:, u]
                            )

                        def store_k(tr, ti=ti):
                            with nc.allow_low_precision(reason="bf16 KT"):
                                nc.vector.tensor_copy(
                                    KT[ti][:],
                                    tr[:].rearrange("p (h d) -> p h d", d=D),
                                )

                        proj_tail(k_ps, cosk_sb, sinkpm_sb, ti, eps_k,
                                  1.0 / D, store_k)
                        flush_tr(2)

                # ---------- attention unit machinery ----------
                class Unit:
                    def __init__(self, qoff, nk, ci, h):
                        self.qoff, self.nk, self.ci, self.h = qoff, nk, ci, h
                        self.scored = 0
                        self.accd = 0
                        self.done = False
                        self.y_ps = None
                        self.d_ps = None
                        self.Es = {}

                    def scores(self):
                        k, h = self.scored, self.h
                        s_ps = psS.tile([128, 512], F32, tag="s", name="s_ps")
                        nc.tensor.matmul(
                            s_ps[:], KT[k][:, h],
                            QTc[self.ci][:, h],
                            start=True, stop=True, skip_group_check=True,
                        )
                        E = ep.tile([128, 512], BF16, tag="E", name="E")
                        nc.scalar.activation(E[:], s_ps[:], EXP)
                        i = k - (self.nk - 4)
                        if i >= 0:
                            # zero E where key row 128*i+r > query col
                            nc.gpsimd.affine_select(
                                out=E[:], in_=E[:],
                                compare_op=mybir.AluOpType.is_ge,
                                fill=0.0, base=-128 * i,
                                pattern=[[1, 512]], channel_multiplier=-1,
                            )
                        self.Es[k] = E
                        self.scored += 1

                    def accum(self):
                        k, h = self.accd, self.h
                        if k == 0:
                            self.y_ps = psY.tile([128, 512], F32, tag="y", name="y_ps")
                            self.d_ps = psD.tile([128, 512], F32, tag="d", name="d_ps")
                        E = self.Es.pop(k)
                        nc.tensor.matmul(
                            self.d_ps[:], ones_mat[:], E[:],
                            start=(k == 0), stop=(k == self.nk - 1),
                            skip_group_check=True,
                        )
                        nc.tensor.matmul(
                            self.y_ps[:], Vn[k][:, h * D : (h + 1) * D], E[:],
                            start=(k == 0), stop=(k == self.nk - 1),
                            skip_group_check=True,
                        )
                        self.accd += 1

                    def epilogue(self):
                        # d_ps has the denominator replicated on all 128
                        # partitions (ones-matrix stationary), so the
                        # reciprocal/multiply run directly on DVE with no
                        # cross-partition broadcast hop
                        recip = rp.tile([128, 512], F32R, tag="recip", name="recip")
                        with nc.allow_low_precision(
                            reason="1/denom as f32r; ~1e-4 uniform scale wobble"
                        ):
                            nc.vector.reciprocal(recip[:], self.d_ps[:])
                        with nc.allow_low_precision(reason="bf16 Ysb; ~4e-3"):
                            nc.vector.tensor_mul(
                                out=Ysb[self.ci][:, self.h], in0=self.y_ps[:],
                                in1=recip[:],
                            )
                        self.y_ps = self.d_ps = None
                        self.done = True

                # small chunk (0, 4s+4) first: fewest key tiles -> earliest
                chunks = [(0, 4 * s + 4, 0), (512, 32 - 4 * s, 1)]
                units = [
                    Unit(qoff, nk, ci, h)
                    for (qoff, nk, ci) in chunks
                    for h in range(HPG)
                ]

                def pump(avail, skew=1, max_iters=None):
                    """Emit attention work whose key tiles (< avail) exist.
                    skew = how many score tiles run ahead of the y/d
                    accumulation (2 once projections stop filling PE)."""
                    iters = 0
                    while True:
                        u = next((x for x in units if not x.done), None)
                        if u is None:
                            return
                        progressed = False
                        target = min(avail, u.nk)
                        while u.scored < target:
                            u.scores()
                            progressed = True
                            iters += 1
                            while u.accd < u.scored - skew:
                                u.accum()
                            if max_iters is not None and iters >= max_iters:
                                while u.accd < u.scored - skew:
                                    u.accum()
                                return
                        if u.scored == u.nk:
                            # prefetch the next unit's first scores so the PE
                            # has work while this unit's last exp lands
                            v = next(
                                (x for x in units if not x.done and x is not u), None
                            )
                            if (
                                v is not None
                                and v.scored == 0
                                and min(avail, v.nk) >= 1
                            ):
                                v.scores()
                                iters += 1
                            while u.accd < u.nk:
                                u.accum()
                            u.epilogue()
                            progressed = True
                        if not progressed:
                            return

                def cproj_chunk(op, psO, ci, qoff):
                    """c_proj for the 4 m-tiles of chunk ci."""
                    for mm in range(4):
                        m = qoff // 128 + mm
                        o_sb = op.tile([128, DIM], F32, tag="o_sb", name="o_sb")
                        for oc in range(2):
                            o_ps = psO.tile([128, GDIM], F32, tag="o_ps", name="o_ps")
                            for h in range(HPG):
                                nc.tensor.matmul(
                                    o_ps[:],
                                    Ysb[ci][:, h, mm * 128 : (mm + 1) * 128],
                                    cproj_sb[:, h, oc * GDIM : (oc + 1) * GDIM],
                                    start=(h == 0), stop=(h == 2),
                                    skip_group_check=True,
                                )
                            nc.scalar.activation(
                                o_sb[:, oc * GDIM : (oc + 1) * GDIM], o_ps[:], COPY
                            )
                        nc.sync.dma_start(y_out[m * 128 : (m + 1) * 128, :], o_sb[:])

                # ---------- the schedule ----------
                emit_q_pair(0)
                emit_kv_pair(0)
                emit_q_pair(1)
                emit_kv_pair(1)
                emit_q_pair(2)
                emit_kv_pair(2)
                emit_q_pair(3)
                emit_kv_pair(3)
                flush_tr(0)  # all Q + first 8 K transposes emitted
                pump(avail=8)
                for tp in range(4, NT // 2):
                    emit_kv_pair(tp)
                    # transposes lag two tiles: KT[2*tp] and KT[2*tp+1] are
                    # still pending, so only tiles 0..2*tp-1 are emitted
                    pump(avail=2 * tp)
                flush_tr(0)
                pump(avail=NT, skew=2, max_iters=6)
                # projection pools close here; their PSUM banks free up for
                # the cproj accumulators while the attention backlog drains
                proj_stack.close()
                with (
                    tc.tile_pool(name=f"op{s}", bufs=2) as op,
                    tc.tile_pool(name=f"psO{s}", bufs=2, space="PSUM") as psO,
                ):
                    cproj_chunk(op, psO, 0, chunks[0][0])
                    pump(avail=NT, skew=2)
                    cproj_chunk(op, psO, 1, chunks[1][0])
                att_stack.close()

            if variant is not None:
                emit_all(variant)
            else:
                with tc.If(sv == 0) as c0:
                    emit_all(0)
                with c0.Else():
                    with tc.If(sv == 1) as c1:
                        emit_all(1)
                    with c1.Else():
                        with tc.If(sv == 2) as c2:
                            emit_all(2)
                        with c2.Else():
                            emit_all(3)

    nc.finalize()
    return nc


def _host_prep(x, ve, qkv_w, lambdas, c_proj_w):
    """Build the 8 per-core input maps (bf16 data path)."""
    import ml_dtypes

    bf16 = ml_dtypes.bfloat16
    x2d = np.ascontiguousarray(x.reshape(T, DIM), dtype=np.float32)
    xT = np.ascontiguousarray(x2d.T)
    ve2 = ve.reshape(T, H, D).astype(np.float32)
    lam0, lam1 = float(lambdas[0]), float(lambdas[1])
    wq_all, wk_all, wv_all = qkv_w[0], qkv_w[1], qkv_w[2]  # [768, 768] each

    t = np.arange(T, dtype=np.float32)
    af = (1.0 / 1024.0) ** np.linspace(0.0, 1.0, 32, dtype=np.float32)
    theta = t[:, None] * af[None, :]
    cos_t = np.cos(theta).astype(np.float32)  # [T, 32]
    sin_t = np.sin(theta).astype(np.float32)
    sin_pm = np.concatenate([sin_t, -sin_t], axis=1)  # [T, 64]

    ident = np.eye(128, dtype=np.float32)
    ones_mat = np.ones((128, 128), dtype=np.float32)

    def pack_xT(m):  # [768, t] -> [t/256, 128, 6, 2, 128]
        t = m.shape[1]
        return np.ascontiguousarray(
            m.reshape(6, 128, t // 256, 2, 128).transpose(2, 1, 0, 3, 4)
        ).astype(bf16)

    def pack_rows(m):  # [t, d] -> [t/256, 128, 2, d]
        t, d = m.shape
        return np.ascontiguousarray(
            m.reshape(t // 256, 2, 128, d).transpose(0, 2, 1, 3)
        ).astype(bf16)

    def pack_tab(m):  # [t, c] -> [128, t/128, c]
        t, c = m.shape
        return np.ascontiguousarray(
            m.reshape(t // 128, 128, c).transpose(1, 0, 2)
        ).astype(bf16)

    xT_packed = pack_xT(xT)
    cosk_p = pack_tab(cos_t)
    sinkpm_p = pack_tab(sin_pm)
    in_maps = []
    for c in range(8):
        g, s = divmod(c, 4)
        hsl = slice(g * GDIM, (g + 1) * GDIM)
        qrows = np.r_[512 * s : 512 * (s + 1), 512 * (7 - s) : 512 * (8 - s)]
        wkv = np.concatenate([wk_all[hsl], lam0 * wv_all[hsl]], axis=0)  # [768, 768]
        in_maps.append(
            {
                "xTt": xT_packed,
                "xqTt": pack_xT(np.ascontiguousarray(xT[:, qrows])),
                "wkv": np.ascontiguousarray(
                    wkv.T.astype(np.float32).reshape(6, 128, 768).transpose(1, 0, 2)
                ).astype(bf16),
                "wq": np.ascontiguousarray(
                    wq_all[hsl].T.astype(np.float32).reshape(6, 128, GDIM).transpose(1, 0, 2)
                ).astype(bf16),
                "vek": pack_rows(
                    (lam1 * ve2[:, g * HPG : (g + 1) * HPG, :]).reshape(T, GDIM)
                ),
                "cosk": cosk_p,
                "sinkpm": sinkpm_p,
                "cosq": pack_tab(np.ascontiguousarray(cos_t[qrows])),
                "sinqpm": pack_tab(np.ascontiguousarray(sin_pm[qrows])),
                "cprojT": np.ascontiguousarray(
                    c_proj_w[:, hsl].T.astype(np.float32).reshape(HPG, 128, DIM).transpose(1, 0, 2)
                ).astype(bf16),
                "ident": ident.astype(bf16),
                "ones_mat": ones_mat.astype(bf16),
                "svar": np.array([[s]], dtype=np.uint32),
            }
        )
    return in_maps


def run(inputs, **run_kwargs):
    if "nc" not in _CACHE:
        _CACHE["nc"] = build_nc()
    nc = _CACHE["nc"]
    in_maps = _host_prep(
        inputs["x"], inputs["ve"], inputs["qkv_w"], inputs["lambdas"], inputs["c_proj_w"]
    )
    res = run_bass_kernel_spmd(nc, in_maps, core_ids=list(range(8)), **run_kwargs)
    out = np.zeros((T, DIM), dtype=np.float32)
    for c, r in enumerate(res.results):
        s = c % 4
        y = r["y"].astype(np.float32)
        out[512 * s : 512 * (s + 1)] += y[:512]
        out[512 * (7 - s) : 512 * (8 - s)] += y[512:]
    return out.reshape(1, T, DIM), res


def kernel(**inputs):
    out, _ = run(inputs)
    return out
